# revision 1
# baseline (speedup 1.0000x reference)
"""Trainium2 Bass kernel for a dense transformer decoder block on 8 NeuronCores.

Sharding (uniform SPMD):
  * tokens: core c owns 512 contiguous tokens — batch c//4, positions
    [512*(c%4), 512*(c%4)+512). All projections, norms and the FFN are
    computed purely locally on those tokens.
  * attention: head-parallel via AllToAll. Each core computes Q/K/V for its
    own tokens (all heads, feature-major, RoPE applied to Q/K), then one
    AllToAll redistributes K,V (and a second one Q) so core c holds heads
    {2c, 2c+1} for ALL 4096 (batch, position) tokens. Causal attention for
    those two heads runs fully on-chip (Q^T/K^T/V all SBUF-resident), and a
    third AllToAll routes the attention output back to token owners for the
    output projection. No all-reduce anywhere.

Layout: activations are feature-major (features on SBUF partitions, tokens on
the free axis) so every matmul is transpose-free: projections compute Y^T
directly (lhsT = weight column block, rhs = X^T), scores are built in S^T
orientation (kv on partitions) which feeds softmax(exp on ScalarE, key-padding
mask folded into the exp bias, causal diagonal handled by an additive band
mask) straight into the attention*V matmul. The softmax denominator is a
ones-column matmul accumulated alongside. RMSNorm statistics are computed with
a Square-activation + ones-matmul (cross-partition reduce on the PE).

Dtypes: float32r (full-rate fp32 PE mode) for all matmuls except ff2, which
runs in bf16 (h is cast at the silu activation so the 8192-wide hidden tensor
fits SBUF-resident and wf2 streams at half bandwidth).
"""
import sys

sys.path.insert(0, '/opt/trn_rl_repo')

import numpy as np
import ml_dtypes

import concourse.bacc as bacc
import concourse.mybir as mybir
from concourse import tile
from concourse.bass_utils import run_bass_kernel_spmd

F32 = mybir.dt.float32
F32R = mybir.dt.float32r
BF16 = mybir.dt.bfloat16
AF = mybir.ActivationFunctionType

D = 2048
H = 16
DH = 128
FF = 8192
B = 2
L = 2048
NCORES = 8
TOK = 512            # tokens per core
NF = D // 128        # 16 feature tiles
NEG = -30000.0
EPS = float(np.finfo(np.float32).eps)
ISQ = 1.0 / float(np.sqrt(DH))
RG = [list(range(NCORES))]


def _build():
    nc = bacc.Bacc("TRN2", target_bir_lowering=False, debug=False,
                   num_devices=NCORES)

    xT = nc.dram_tensor("xT", [D, TOK], F32, kind="ExternalInput")
    wq = nc.dram_tensor("wq", [D, D], F32R, kind="ExternalInput")
    wk = nc.dram_tensor("wk", [D, D], F32R, kind="ExternalInput")
    wv = nc.dram_tensor("wv", [D, D], F32R, kind="ExternalInput")
    wo = nc.dram_tensor("wo", [D, D], F32R, kind="ExternalInput")
    wf1 = nc.dram_tensor("wf1", [D, FF], F32R, kind="ExternalInput")
    wf2 = nc.dram_tensor("wf2", [FF, D], BF16, kind="ExternalInput")
    ropeC = nc.dram_tensor("ropeC", [DH, TOK], F32, kind="ExternalInput")
    ropeS2 = nc.dram_tensor("ropeS2", [DH, TOK], F32, kind="ExternalInput")
    band = nc.dram_tensor("band", [128, 896], F32, kind="ExternalInput")
    mbias = nc.dram_tensor("mbias", [128, 2 * H], F32, kind="ExternalInput")
    onesd = nc.dram_tensor("onesd", [128, 1], F32R, kind="ExternalInput")
    outT = nc.dram_tensor("outT", [D, TOK], F32, kind="ExternalOutput")

    # internal DRAM: AllToAll bounce buffers + spills
    kvin = nc.dram_tensor("kvin", [2 * D, TOK], F32R)
    kvout = nc.dram_tensor("kvout", [2 * D, TOK], F32R)
    qin = nc.dram_tensor("qin", [D, TOK], F32R)
    qout = nc.dram_tensor("qout", [D, TOK], F32R)
    oin = nc.dram_tensor("oin", [D, TOK], F32R)
    oout = nc.dram_tensor("oout", [D, TOK], F32R)
    x2d = nc.dram_tensor("x2d", [D, TOK], F32)

    with tile.TileContext(nc) as tc:
        with (
            tc.tile_pool(name="const", bufs=1) as cp,
            tc.tile_pool(name="small", bufs=1) as sp,
        ):
            cosT = cp.tile([DH, TOK], F32)
            sin2 = cp.tile([DH, TOK], F32)
            bandT = cp.tile([128, 896], F32)
            mbT = cp.tile([128, 2 * H], F32)
            onec = cp.tile([128, 1], F32R)
            epsc = cp.tile([1, 1], F32)
            nc.scalar.dma_start(cosT[:], ropeC[:])
            nc.scalar.dma_start(sin2[:], ropeS2[:])
            nc.scalar.dma_start(bandT[:], band[:])
            nc.scalar.dma_start(mbT[:], mbias[:])
            nc.scalar.dma_start(onec[:], onesd[:])
            nc.gpsimd.memset(epsc[:], EPS)

            rsB = sp.tile([128, TOK], F32)
            rowS = sp.tile([1, TOK], F32)
            rowR = sp.tile([1, TOK], F32)

            def rmsnorm_rs(ssq_ps):
                nc.scalar.activation(rowS[:], ssq_ps[:], AF.Sqrt,
                                     bias=epsc[:], scale=1.0 / D)
                nc.vector.reciprocal(rowR[:], rowS[:])
                nc.gpsimd.partition_broadcast(rsB[:], rowR[:])

            # ========== Phase 1: norm1, K^T, V, Q^T, AllToAlls ==========
            with (
                tc.tile_pool(name="m1", bufs=1) as m1,
                tc.tile_pool(name="ps1", bufs=3, space="PSUM") as ps1,
                tc.tile_pool(name="psr", bufs=1, space="PSUM") as psr,
            ):
                xt = m1.tile([128, NF * TOK], F32, tag="t1")
                nc.scalar.dma_start(
                    xt[:].rearrange("p (i c) -> p i c", i=NF),
                    xT[:].rearrange("(i p) c -> p i c", p=128))

                ssq = psr.tile([1, TOK], F32, tag="row")
                for i in range(NF):
                    sq = sp.tile([128, TOK], F32R, tag="sq", bufs=2)
                    nc.scalar.activation(sq[:], xt[:, i * TOK:(i + 1) * TOK],
                                         AF.Square)
                    nc.tensor.matmul(ssq[:], onec[:], sq[:],
                                     start=(i == 0), stop=(i == NF - 1))
                rmsnorm_rs(ssq)
                xnt = m1.tile([128, NF * TOK], F32R, tag="xn")
                for i in range(NF):
                    nc.vector.tensor_mul(xnt[:, i * TOK:(i + 1) * TOK],
                                         xt[:, i * TOK:(i + 1) * TOK], rsB[:])

                def proj_T(wten, out_tile, rope):
                    """out_tile[:, o*TOK:] = head-tile o of (xn @ w)^T."""
                    for o in range(NF):
                        wc = m1.tile([128, NF * 128], F32R, tag="wcol",
                                     bufs=3)
                        nc.sync.dma_start(
                            wc[:].rearrange("p (i m) -> p i m", i=NF),
                            wten[:, o * 128:(o + 1) * 128]
                            .rearrange("(i p) m -> p i m", p=128))
                        acc = ps1.tile([128, TOK], F32, tag="big")
                        for i in range(NF):
                            nc.tensor.matmul(
                                acc[:], wc[:, i * 128:(i + 1) * 128],
                                xnt[:, i * TOK:(i + 1) * TOK],
                                start=(i == 0), stop=(i == NF - 1))
                        dst = out_tile[:, o * TOK:(o + 1) * TOK]
                        if rope:
                            tmp = sp.tile([128, TOK], F32R, tag="rtmp",
                                          bufs=2)
                            nc.vector.tensor_mul(tmp[0:64, :], acc[64:128, :],
                                                 sin2[0:64, :])
                            nc.vector.tensor_mul(tmp[64:128, :], acc[0:64, :],
                                                 sin2[64:128, :])
                            nc.vector.tensor_mul(dst, acc[:], cosT[:])
                            nc.vector.tensor_add(dst, dst, tmp[:])
                        else:
                            nc.vector.tensor_copy(dst, acc[:])

                # K^T (roped) — reuses xt's slot (xt is dead after norm1)
                kt = m1.tile([128, NF * TOK], F32R, tag="t1")
                proj_T(wk, kt, rope=True)

                # V (token-major), 256-wide feature chunks
                vt = m1.tile([128, 4 * D], F32R, tag="t2")
                for fo in range(8):
                    wvc = m1.tile([128, NF * 256], F32R, tag="wv", bufs=2)
                    nc.scalar.dma_start(
                        wvc[:].rearrange("p (i m) -> p i m", i=NF),
                        wv[:, fo * 256:(fo + 1) * 256]
                        .rearrange("(i p) m -> p i m", p=128))
                    for to in range(4):
                        acc = ps1.tile([128, 256], F32, tag="big")
                        for i in range(NF):
                            nc.tensor.matmul(
                                acc[:],
                                xnt[:, i * TOK + to * 128:
                                    i * TOK + (to + 1) * 128],
                                wvc[:, i * 256:(i + 1) * 256],
                                start=(i == 0), stop=(i == NF - 1))
                        nc.vector.tensor_copy(
                            vt[:, to * D + fo * 256:to * D + (fo + 1) * 256],
                            acc[:])

                # bounce K+V bundle, kick AllToAll #1
                kv4 = kvin.ap().rearrange("(j q d) (t f) -> j d q t f",
                                          j=NCORES, q=4, d=128, t=2, f=256)
                vtv = vt[:].rearrange("p (t1 t2 j f) -> p t1 j t2 f",
                                      t1=2, t2=2, j=NCORES, f=256)
                for j in range(NCORES):
                    nc.gpsimd.dma_start(
                        kv4[j, :, 0:2, :, :]
                        .rearrange("d q t f -> d q (t f)"),
                        kt[:].rearrange("p (o c) -> p o c", o=NF)
                        [:, 2 * j:2 * j + 2, :])
                    for t1 in range(2):
                        nc.gpsimd.dma_start(
                            kv4[j, :, 2:4, t1, :],
                            vtv[:, t1, j, :, :])
                nc.gpsimd.collective_compute(
                    "AllToAll", mybir.AluOpType.bypass, replica_groups=RG,
                    ins=[kvin.ap().opt()], outs=[kvout.ap().opt()])

                # Q^T (roped), bounce, AllToAll #2 — reuses vt's slot
                qt = m1.tile([128, NF * TOK], F32R, tag="t2")
                proj_T(wq, qt, rope=True)
                qiv = qin.ap().rearrange("(j s d) c -> j d s c", j=NCORES,
                                         s=2, d=128)
                for j in range(NCORES):
                    nc.gpsimd.dma_start(
                        qiv[j], qt[:].rearrange("p (o c) -> p o c", o=NF)
                        [:, 2 * j:2 * j + 2, :])
                nc.gpsimd.collective_compute(
                    "AllToAll", mybir.AluOpType.bypass, replica_groups=RG,
                    ins=[qin.ap().opt()], outs=[qout.ap().opt()])

            # ============ Phase 2: attention (heads 2c, 2c+1) ============
            with (
                tc.tile_pool(name="m2", bufs=1) as m2,
                tc.tile_pool(name="ps_s", bufs=2, space="PSUM") as ps_s,
                tc.tile_pool(name="ps_av", bufs=2, space="PSUM") as ps_av,
                tc.tile_pool(name="ps_dn", bufs=2, space="PSUM") as ps_dn,
            ):
                ksb = m2.tile([128, 2 * 4096], F32R)   # [dh, hh, (b,pos)]
                vsb = m2.tile([128, 32 * 256], F32R)   # [kv%128, tile, feat]
                qsb = m2.tile([128, 2 * 4096], F32R)
                osb = m2.tile([128, 2 * 4096], F32R)
                kv4o = kvout.ap().rearrange("(j q d) (t f) -> j d q t f",
                                            j=NCORES, q=4, d=128, t=2, f=256)
                ksbv = ksb[:].rearrange("d (h j c) -> d h j c", h=2, j=NCORES)
                vsbv = vsb[:].rearrange(
                    "p (jj t1 t2 f) -> p jj t1 t2 f", jj=NCORES, t1=2, t2=2,
                    f=256)
                qsbv = qsb[:].rearrange("d (h j c) -> d h j c", h=2, j=NCORES)
                qov = qout.ap().rearrange("(j s d) c -> j d s c", j=NCORES,
                                          s=2, d=128)
                for j in range(NCORES):
                    nc.scalar.dma_start(
                        ksbv[:, :, j, :],
                        kv4o[j, :, 0:2, :, :]
                        .rearrange("d q t f -> d q (t f)"))
                    for t1 in range(2):
                        nc.scalar.dma_start(
                            vsbv[:, j, t1, :, :], kv4o[j, :, 2:4, t1, :])
                    nc.scalar.dma_start(qsbv[:, :, j, :], qov[j])

                for b in range(B):
                    for hh in range(2):
                        for q4 in range(4):
                            qs = qsb[:, hh * 4096 + b * 2048 + q4 * 512:
                                     hh * 4096 + b * 2048 + (q4 + 1) * 512]
                            ng = 4 * q4 + 4
                            av = ps_av.tile([128, 512], F32, tag="av")
                            dn = ps_dn.tile([1, 512], F32, tag="dn")
                            for g in range(ng):
                                st = ps_s.tile([128, 512], F32, tag="s")
                                nc.tensor.matmul(
                                    st[:],
                                    ksb[:, hh * 4096 + b * 2048 + g * 128:
                                        hh * 4096 + b * 2048 + (g + 1) * 128],
                                    qs, start=True, stop=True)
                                if g >= 4 * q4:
                                    r = (g - 4 * q4) * 128
                                    nc.vector.tensor_add(
                                        st[:], st[:],
                                        bandT[:, 384 - r:896 - r])
                                pt = sp.tile([128, 512], F32R, tag="pt",
                                             bufs=3)
                                nc.scalar.activation(
                                    pt[:], st[:], AF.Exp,
                                    bias=mbT[:, b * H + g:b * H + g + 1],
                                    scale=ISQ)
                                nc.tensor.matmul(dn[:], onec[:], pt[:],
                                                 start=(g == 0),
                                                 stop=(g == ng - 1))
                                nc.tensor.matmul(
                                    av[:],
                                    vsb[:, (b * H + g) * 256 + hh * 128:
                                        (b * H + g) * 256 + (hh + 1) * 128],
                                    pt[:], start=(g == 0), stop=(g == ng - 1))
                            dnr = sp.tile([1, 512], F32, tag="dnr", bufs=2)
                            nc.vector.reciprocal(dnr[:], dn[:])
                            rdB = sp.tile([128, 512], F32, tag="rdB", bufs=2)
                            nc.gpsimd.partition_broadcast(rdB[:], dnr[:])
                            nc.vector.tensor_mul(
                                osb[:, hh * 4096 + b * 2048 + q4 * 512:
                                    hh * 4096 + b * 2048 + (q4 + 1) * 512],
                                av[:], rdB[:])

                oiv = oin.ap().rearrange("(j s d) c -> j d s c", j=NCORES,
                                         s=2, d=128)
                osv = osb[:].rearrange("d (h j c) -> d h j c", h=2, j=NCORES)
                for j in range(NCORES):
                    nc.gpsimd.dma_start(oiv[j], osv[:, :, j, :])
                nc.gpsimd.collective_compute(
                    "AllToAll", mybir.AluOpType.bypass, replica_groups=RG,
                    ins=[oin.ap().opt()], outs=[oout.ap().opt()])

            # ======= Phase 3: O-projection + residual + norm2 stats ======
            with (
                tc.tile_pool(name="m3", bufs=1) as m3,
                tc.tile_pool(name="ps3", bufs=2, space="PSUM") as ps3,
                tc.tile_pool(name="psr3", bufs=1, space="PSUM") as psr3,
            ):
                ao = m3.tile([128, NF * TOK], F32R)  # attnT, all heads
                nc.scalar.dma_start(
                    ao[:].rearrange("p (i c) -> p i c", i=NF),
                    oout.ap().rearrange("(i p) c -> p i c", p=128))
                ssq2 = psr3.tile([1, TOK], F32, tag="row")
                for o in range(NF):
                    wc = m3.tile([128, NF * 128], F32R, tag="wocol", bufs=3)
                    nc.sync.dma_start(
                        wc[:].rearrange("p (i m) -> p i m", i=NF),
                        wo[:, o * 128:(o + 1) * 128]
                        .rearrange("(i p) m -> p i m", p=128))
                    acc = ps3.tile([128, TOK], F32, tag="big")
                    for i in range(NF):
                        nc.tensor.matmul(acc[:], wc[:, i * 128:(i + 1) * 128],
                                         ao[:, i * TOK:(i + 1) * TOK],
                                         start=(i == 0), stop=(i == NF - 1))
                    xsl = m3.tile([128, TOK], F32, tag="xsl", bufs=2)
                    nc.scalar.dma_start(xsl[:], xT[o * 128:(o + 1) * 128, :])
                    x2sl = m3.tile([128, TOK], F32, tag="x2sl", bufs=2)
                    nc.vector.tensor_add(x2sl[:], xsl[:], acc[:])
                    nc.scalar.dma_start(x2d[o * 128:(o + 1) * 128, :],
                                        x2sl[:])
                    sq = sp.tile([128, TOK], F32R, tag="sq", bufs=2)
                    nc.scalar.activation(sq[:], x2sl[:], AF.Square)
                    nc.tensor.matmul(ssq2[:], onec[:], sq[:],
                                     start=(o == 0), stop=(o == NF - 1))
                rmsnorm_rs(ssq2)

            # ==================== Phase 4: norm2 + FFN ===================
            with (
                tc.tile_pool(name="m4", bufs=1) as m4,
                tc.tile_pool(name="ps4", bufs=3, space="PSUM") as ps4,
            ):
                xn2 = m4.tile([128, NF * TOK], F32R)
                for i in range(NF):
                    xsl = m4.tile([128, TOK], F32, tag="xsl", bufs=2)
                    nc.scalar.dma_start(xsl[:], x2d[i * 128:(i + 1) * 128, :])
                    nc.vector.tensor_mul(xn2[:, i * TOK:(i + 1) * TOK],
                                         xsl[:], rsB[:])
                # ff1 + silu -> h (bf16, SBUF resident)
                h = m4.tile([128, 64 * TOK], BF16)
                for o in range(FF // 128):
                    wc = m4.tile([128, NF * 128], F32R, tag="wf1c", bufs=3)
                    nc.sync.dma_start(
                        wc[:].rearrange("p (i m) -> p i m", i=NF),
                        wf1[:, o * 128:(o + 1) * 128]
                        .rearrange("(i p) m -> p i m", p=128))
                    acc = ps4.tile([128, TOK], F32, tag="big")
                    for i in range(NF):
                        nc.tensor.matmul(acc[:], wc[:, i * 128:(i + 1) * 128],
                                         xn2[:, i * TOK:(i + 1) * TOK],
                                         start=(i == 0), stop=(i == NF - 1))
                    nc.scalar.activation(h[:, o * TOK:(o + 1) * TOK], acc[:],
                                         AF.Silu)
                # ff2 (bf16) + residual -> outT
                for o in range(NF):
                    wc2 = m4.tile([128, 64 * 128], BF16, tag="wf2c", bufs=2)
                    nc.scalar.dma_start(
                        wc2[:].rearrange("p (k m) -> p k m", k=64),
                        wf2[:, o * 128:(o + 1) * 128]
                        .rearrange("(k p) m -> p k m", p=128))
                    acc = ps4.tile([128, TOK], F32, tag="big")
                    for k in range(64):
                        nc.tensor.matmul(acc[:], wc2[:, k * 128:(k + 1) * 128],
                                         h[:, k * TOK:(k + 1) * TOK],
                                         start=(k == 0), stop=(k == 63))
                    xsl = m4.tile([128, TOK], F32, tag="xsl", bufs=2)
                    nc.scalar.dma_start(xsl[:], x2d[o * 128:(o + 1) * 128, :])
                    osl = m4.tile([128, TOK], F32, tag="osl", bufs=2)
                    nc.vector.tensor_add(osl[:], xsl[:], acc[:])
                    nc.sync.dma_start(outT[o * 128:(o + 1) * 128, :], osl[:])

    nc.compile()
    return nc


_COMPILED = None


def _prep_inmaps(x, rope_cos, rope_sin, mask, w_norm1, w_norm2, wq, wk, wv,
                 wo, w_ff1, w_ff2):
    x = np.asarray(x, np.float32)
    cos = np.asarray(rope_cos, np.float32)
    sin = np.asarray(rope_sin, np.float32)
    mask = np.asarray(mask)
    wn1 = np.asarray(w_norm1, np.float32)
    wn2 = np.asarray(w_norm2, np.float32)

    wqn = np.ascontiguousarray(wn1[:, None] * np.asarray(wq, np.float32))
    wkn = np.ascontiguousarray(wn1[:, None] * np.asarray(wk, np.float32))
    wvn = np.ascontiguousarray(wn1[:, None] * np.asarray(wv, np.float32))
    won = np.ascontiguousarray(np.asarray(wo, np.float32))
    wf1n = np.ascontiguousarray(wn2[:, None] * np.asarray(w_ff1, np.float32))
    wf2b = np.asarray(w_ff2, np.float32).astype(ml_dtypes.bfloat16)

    # causal band mask: band[row, cc] = 0 iff cc >= row + 384
    cc = np.arange(896)[None, :]
    rr = np.arange(128)[:, None]
    band = np.where(cc >= rr + 384, 0.0, NEG).astype(np.float32)
    # key-padding mask bias, [128, 2*H]: col b*16+g <- kv pos 128g+p
    mb = np.where(mask != 0, 0.0, NEG).astype(np.float32)  # [B, L]
    mbias = np.ascontiguousarray(
        mb.reshape(B, H, 128).transpose(2, 0, 1).reshape(128, B * H))

    in_maps = []
    for c in range(NCORES):
        b = c // 4
        lo = 512 * (c % 4)
        pos = slice(lo, lo + TOK)
        s = sin[pos].T.copy()
        s2 = np.concatenate([-s[:64], s[64:]], axis=0)
        in_maps.append({
            "xT": np.ascontiguousarray(x[b, pos].T),
            "wq": wqn, "wk": wkn, "wv": wvn, "wo": won,
            "wf1": wf1n, "wf2": wf2b,
            "ropeC": np.ascontiguousarray(cos[pos].T),
            "ropeS2": np.ascontiguousarray(s2),
            "band": band, "mbias": mbias,
            "onesd": np.ones((128, 1), np.float32),
        })
    return in_maps


def _assemble(res):
    out = np.empty((B, L, D), np.float32)
    for c in range(NCORES):
        b = c // 4
        lo = 512 * (c % 4)
        out[b, lo:lo + TOK, :] = res.results[c]["outT"].T
    return out


def kernel(**inputs):
    global _COMPILED
    if _COMPILED is None:
        _COMPILED = _build()
    in_maps = _prep_inmaps(**inputs)
    res = run_bass_kernel_spmd(_COMPILED, in_maps, list(range(NCORES)))
    return _assemble(res)


def timed_run(**inputs):
    """Run with NTFF profiling; returns (exec_time_ns, BassKernelResults)."""
    global _COMPILED
    if _COMPILED is None:
        _COMPILED = _build()
    in_maps = _prep_inmaps(**inputs)
    res = run_bass_kernel_spmd(_COMPILED, in_maps, list(range(NCORES)),
                               trace=True)
    return res.exec_time_ns, res



# revision 16
# speedup vs baseline: 1.3144x; 1.3144x over previous
"""Trainium2 Bass kernel for a dense transformer decoder block on 8 NeuronCores.

Sharding (uniform SPMD, v2 — AllGather design):
  * tokens: core c owns 512 tokens of batch b=c//4: the two 256-position
    stripes {256*c4, 256*(7-c4)} (c4=c%4). The stripe pairing balances causal
    attention work exactly (18 kv-blocks of 128 per head on every core).
  * attention is query-sharded: each core attends its OWN 512 queries over
    ALL heads. K and V (computed locally per token owner, rope applied to K)
    are AllGathered within the 4-core batch group — the only collectives in
    the kernel, each fired right after its producing projection so it hides
    under the next projection's compute. Q, attention output, the
    O-projection, residual, norm2 and the FFN are all local. No AllToAll,
    no all-reduce.
  * SPMD uniformity: every core runs the same padded kv-prefix length per
    query slot (8 blocks for slot 0, 16 for slot 1). Per-core mask *data*
    (multiplicative {0,1} on the exp output: causal diagonal + padding,
    plus the key-padding bias inside the exp) zeroes blocks beyond that
    core's real causal extent.

Everything runs in bf16 on the PE (fp32 PSUM accumulation): 2x less HBM
traffic and collective payload than fp32, and bf16 enables fast-weight-load
so LDWEIGHTS overlaps matmuls. Weights are host-packed into the exact
column-block tile layout so every weight DMA is one contiguous 2D transfer.
Softmax statistics (exp on ScalarE, denominator via ones-matmul) and RMSNorm
statistics (Square + ones-matmul) ride on otherwise idle engines.
"""
import sys

sys.path.insert(0, '/opt/trn_rl_repo')

import numpy as np
import ml_dtypes

import concourse.bacc as bacc
import concourse.mybir as mybir
from concourse import tile
from concourse.bass_utils import run_bass_kernel_spmd

F32 = mybir.dt.float32
F32R = mybir.dt.float32r
BF16 = mybir.dt.bfloat16
AF = mybir.ActivationFunctionType

D = 2048
H = 16
DH = 128
FF = 8192
B = 2
L = 2048
NCORES = 8
TOK = 512            # tokens per core
NF = D // 128        # 16 feature chunks
NEG = -30000.0
EPS = float(np.finfo(np.float32).eps)
ISQ = 1.0 / float(np.sqrt(DH))
RG4 = [[0, 1, 2, 3], [4, 5, 6, 7]]


def _kv_loc(j):
    """kv 128-block j (absolute) -> (rank, slot, half) in gathered buffers."""
    p, hf = j // 2, j % 2
    r = p if p < 4 else 7 - p
    s = 0 if p < 4 else 1
    return r, s, hf


def _build():
    nc = bacc.Bacc("TRN2", target_bir_lowering=False, debug=False,
                   num_devices=NCORES)

    xT = nc.dram_tensor("xT", [D, TOK], F32, kind="ExternalInput")
    wqp = nc.dram_tensor("wqp", [NF * 128, D], BF16, kind="ExternalInput")
    wkp = nc.dram_tensor("wkp", [NF * 128, D], BF16, kind="ExternalInput")
    wvp = nc.dram_tensor("wvp", [8 * 128, 4096], BF16, kind="ExternalInput")
    wop = nc.dram_tensor("wop", [NF * 128, D], BF16, kind="ExternalInput")
    wf1p = nc.dram_tensor("wf1p", [64 * 128, D], BF16, kind="ExternalInput")
    wf2p = nc.dram_tensor("wf2p", [NF * 128, FF], BF16, kind="ExternalInput")
    ropeC = nc.dram_tensor("ropeC", [DH, TOK], F32, kind="ExternalInput")
    ropeS2 = nc.dram_tensor("ropeS2", [DH, TOK], F32, kind="ExternalInput")
    maskM = nc.dram_tensor("maskM", [128, 16 * 256], BF16,
                           kind="ExternalInput")
    mbias = nc.dram_tensor("mbias", [128, 16], F32, kind="ExternalInput")
    onesf = nc.dram_tensor("onesf", [128, 1], F32R, kind="ExternalInput")
    onesb = nc.dram_tensor("onesb", [128, 1], BF16, kind="ExternalInput")
    outT = nc.dram_tensor("outT", [D, TOK], F32, kind="ExternalOutput")

    # internal DRAM: AllGather bounce buffers
    kgin = nc.dram_tensor("kgin", [D, TOK], BF16)
    kgout = nc.dram_tensor("kgout", [4 * D, TOK], BF16)
    vgin = nc.dram_tensor("vgin", [TOK, D], BF16)
    vgout = nc.dram_tensor("vgout", [4 * TOK, D], BF16)

    with tile.TileContext(nc) as tc:
        with (
            tc.tile_pool(name="const", bufs=1) as cp,
            tc.tile_pool(name="small", bufs=1) as sp,
            tc.tile_pool(name="mq", bufs=1) as mq,
        ):
            one_r = cp.tile([128, 1], F32R)
            one_b = cp.tile([128, 1], BF16)
            epsc = cp.tile([1, 1], F32)
            nc.scalar.dma_start(one_r[:], onesf[:])
            nc.scalar.dma_start(one_b[:], onesb[:])
            nc.gpsimd.memset(epsc[:], EPS)

            rsB = sp.tile([128, TOK], F32)
            rowS = sp.tile([1, TOK], F32)
            rowR = sp.tile([1, TOK], F32)
            qt = mq.tile([128, NF * TOK], BF16, tag="qt")

            def rmsnorm_rs(ssq_ps):
                # rowR = 1/sqrt(ssq/D + eps), broadcast to 128 partitions
                nc.scalar.activation(rowS[:], ssq_ps[:], AF.Sqrt,
                                     bias=epsc[:], scale=1.0 / D)
                nc.vector.reciprocal(rowR[:], rowS[:])
                nc.gpsimd.partition_broadcast(rsB[:], rowR[:])

            # ====== Phase A-D: norm1, K/V/Q projections, AllGathers ======
            with (
                tc.tile_pool(name="m1", bufs=1) as m1,
                tc.tile_pool(name="ps1", bufs=3, space="PSUM") as ps1,
                tc.tile_pool(name="psr", bufs=1, space="PSUM") as psr,
            ):
                cosT = m1.tile([DH, TOK], F32, tag="cosT")
                sin2 = m1.tile([DH, TOK], F32, tag="sin2")
                nc.scalar.dma_start(cosT[:], ropeC[:])
                nc.scalar.dma_start(sin2[:], ropeS2[:])

                xt = m1.tile([128, NF * TOK], F32, tag="xt")
                nc.scalar.dma_start(
                    xt[:].rearrange("p (i c) -> p i c", i=NF),
                    xT[:].rearrange("(i p) c -> p i c", p=128))

                ssq = psr.tile([1, TOK], F32, tag="row")
                for i in range(NF):
                    sq = sp.tile([128, TOK], F32R, tag="sq", bufs=1)
                    nc.scalar.activation(sq[:], xt[:, i * TOK:(i + 1) * TOK],
                                         AF.Square)
                    nc.tensor.matmul(ssq[:], one_r[:], sq[:],
                                     start=(i == 0), stop=(i == NF - 1))
                rmsnorm_rs(ssq)
                xnt = m1.tile([128, NF * TOK], BF16, tag="xn")
                for i in range(NF):
                    nc.vector.tensor_mul(xnt[:, i * TOK:(i + 1) * TOK],
                                         xt[:, i * TOK:(i + 1) * TOK], rsB[:])

                def proj_T(wten, out_tile, rope):
                    """out_tile[:, o*TOK:] = feature-block o of (xn @ w)^T."""
                    for o in range(NF):
                        wc = m1.tile([128, D], BF16, tag="wcol", bufs=3)
                        nc.sync.dma_start(wc[:],
                                          wten[o * 128:(o + 1) * 128, :])
                        acc = ps1.tile([128, TOK], F32, tag="big")
                        for i in range(NF):
                            nc.tensor.matmul(
                                acc[:], wc[:, i * 128:(i + 1) * 128],
                                xnt[:, i * TOK:(i + 1) * TOK],
                                start=(i == 0), stop=(i == NF - 1))
                        dst = out_tile[:, o * TOK:(o + 1) * TOK]
                        if rope:
                            tmp = sp.tile([128, TOK], F32, tag="rtmp",
                                          bufs=1)
                            tmc = sp.tile([128, TOK], F32, tag="rtmc",
                                          bufs=1)
                            nc.vector.tensor_mul(tmp[0:64, :], acc[64:128, :],
                                                 sin2[0:64, :])
                            nc.vector.tensor_mul(tmp[64:128, :], acc[0:64, :],
                                                 sin2[64:128, :])
                            nc.vector.tensor_mul(tmc[:], acc[:], cosT[:])
                            nc.vector.tensor_add(dst, tmc[:], tmp[:])
                        else:
                            nc.vector.tensor_copy(dst, acc[:])
                        yield o

                # K^T (roped) -> kgin chunks -> AllGather
                kt = m1.tile([128, NF * TOK], BF16, tag="kt")
                for o in proj_T(wkp, kt, rope=True):
                    nc.gpsimd.dma_start(kgin.ap()[o * 128:(o + 1) * 128, :],
                                        kt[:, o * TOK:(o + 1) * TOK])
                nc.gpsimd.collective_compute(
                    "AllGather", mybir.AluOpType.bypass, replica_groups=RG4,
                    ins=[kgin.ap().opt()], outs=[kgout.ap().opt()])

                # V (token-major) -> vgin -> AllGather
                vt = m1.tile([128, 4 * D], BF16, tag="vt")
                for fo in range(8):
                    wvc = m1.tile([128, NF * 256], BF16, tag="wv", bufs=2)
                    nc.sync.dma_start(wvc[:],
                                      wvp[fo * 128:(fo + 1) * 128, :])
                    for to in range(4):
                        acc = ps1.tile([128, 256], F32, tag="vacc", bufs=3)
                        for i in range(NF):
                            nc.tensor.matmul(
                                acc[:],
                                xnt[:, i * TOK + to * 128:
                                    i * TOK + (to + 1) * 128],
                                wvc[:, i * 256:(i + 1) * 256],
                                start=(i == 0), stop=(i == NF - 1))
                        nc.vector.tensor_copy(
                            vt[:, to * D + fo * 256:to * D + (fo + 1) * 256],
                            acc[:])
                for to in range(4):
                    nc.gpsimd.dma_start(vgin.ap()[to * 128:(to + 1) * 128, :],
                                        vt[:, to * D:(to + 1) * D])
                nc.gpsimd.collective_compute(
                    "AllGather", mybir.AluOpType.bypass, replica_groups=RG4,
                    ins=[vgin.ap().opt()], outs=[vgout.ap().opt()])

                # Q^T (roped), stays local in qt
                for _ in proj_T(wqp, qt, rope=True):
                    pass

            # ============ Phase E: attention (local queries) =============
            with tc.tile_pool(name="mo", bufs=1) as mo:
                osb = mo.tile([128, NF * TOK], BF16, tag="osb")
                with (
                    tc.tile_pool(name="m2", bufs=1) as m2,
                    tc.tile_pool(name="ps_s", bufs=3, space="PSUM") as ps_s,
                    tc.tile_pool(name="ps_av", bufs=2, space="PSUM") as ps_av,
                    tc.tile_pool(name="ps_dn", bufs=2, space="PSUM") as ps_dn,
                ):
                    mkT = m2.tile([128, 16 * 256], BF16, tag="mkT")
                    mbT = m2.tile([128, 16], F32, tag="mbT")
                    nc.scalar.dma_start(mkT[:], maskM[:])
                    nc.scalar.dma_start(mbT[:], mbias[:])

                    # ksb view: [dh, h, r, c(512)] ; vsb view: [p, G, f]
                    ksb = m2.tile([128, 16 * D], BF16, tag="ksb")
                    vsb = m2.tile([128, 16 * D], BF16, tag="vsb")
                    ksbv = ksb[:].rearrange("d (h r c) -> d h r c", h=16, r=4)
                    vsbv = vsb[:].rearrange("p (g f) -> p g f", g=16)
                    eng4 = [nc.scalar, nc.sync, nc.gpsimd, nc.scalar]
                    for r in range(4):
                        eng4[r].dma_start(
                            ksbv[:, :, r, :],
                            kgout.ap()[r * D:(r + 1) * D, :]
                            .rearrange("(h d) c -> d h c", d=128))
                        eng4[r].dma_start(
                            vsbv[:, r * 4:(r + 1) * 4, :],
                            vgout.ap()[r * TOK:(r + 1) * TOK, :]
                            .rearrange("(g p) f -> p g f", p=128))

                    for h in range(H):
                        for s in range(2):
                            qs = qt[:, h * TOK + s * 256:
                                    h * TOK + (s + 1) * 256]
                            n_it = 8 if s == 0 else 16
                            av = ps_av.tile([128, 256], F32, tag="av")
                            dn = ps_dn.tile([1, 256], F32, tag="dn")
                            for t in range(n_it):
                                r, sl, hf = _kv_loc(t)
                                kcol = h * D + r * TOK + sl * 256 + hf * 128
                                st = ps_s.tile([128, 256], F32, tag="st")
                                nc.tensor.matmul(
                                    st[:], ksb[:, kcol:kcol + 128], qs,
                                    start=True, stop=True)
                                pt = sp.tile([128, 256], BF16, tag="pt",
                                             bufs=3)
                                nc.scalar.activation(
                                    pt[:], st[:], AF.Exp,
                                    bias=mbT[:, t:t + 1], scale=ISQ)
                                if s == 0 or t >= 8:
                                    nc.vector.tensor_mul(
                                        pt[:], pt[:],
                                        mkT[:, t * 256:(t + 1) * 256])
                                G = r * 4 + sl * 2 + hf
                                nc.tensor.matmul(dn[:], one_b[:], pt[:],
                                                 start=(t == 0),
                                                 stop=(t == n_it - 1))
                                nc.tensor.matmul(
                                    av[:], vsb[:, G * D + h * 128:
                                               G * D + (h + 1) * 128],
                                    pt[:], start=(t == 0),
                                    stop=(t == n_it - 1))
                            dnr = sp.tile([1, 256], F32, tag="dnr", bufs=2)
                            nc.scalar.activation(dnr[:], dn[:], AF.Copy)
                            dnB = sp.tile([128, 256], F32, tag="dnB", bufs=2)
                            nc.gpsimd.partition_broadcast(dnB[:], dnr[:])
                            rdB = sp.tile([128, 256], F32, tag="rdB", bufs=2)
                            nc.vector.reciprocal(rdB[:], dnB[:])
                            nc.vector.tensor_mul(
                                osb[:, h * TOK + s * 256:
                                    h * TOK + (s + 1) * 256],
                                av[:], rdB[:])

                # ===== Phase F: O-projection + residual + norm2 stats ====
                with (
                    tc.tile_pool(name="m3", bufs=1) as m3,
                    tc.tile_pool(name="ps3", bufs=3, space="PSUM") as ps3,
                    tc.tile_pool(name="psr3", bufs=1, space="PSUM") as psr3,
                ):
                    x2 = m3.tile([128, NF * TOK], F32, tag="x2")
                    ssq2 = psr3.tile([1, TOK], F32, tag="row")
                    for o in range(NF):
                        wc = m3.tile([128, D], BF16, tag="wocol", bufs=2)
                        nc.sync.dma_start(wc[:],
                                          wop[o * 128:(o + 1) * 128, :])
                        acc = ps3.tile([128, TOK], F32, tag="big")
                        for i in range(NF):
                            nc.tensor.matmul(
                                acc[:], wc[:, i * 128:(i + 1) * 128],
                                osb[:, i * TOK:(i + 1) * TOK],
                                start=(i == 0), stop=(i == NF - 1))
                        xsl = m3.tile([128, TOK], F32, tag="xsl", bufs=2)
                        nc.scalar.dma_start(xsl[:],
                                            xT[o * 128:(o + 1) * 128, :])
                        nc.vector.tensor_add(x2[:, o * TOK:(o + 1) * TOK],
                                             xsl[:], acc[:])
                        sq = sp.tile([128, TOK], F32R, tag="sq", bufs=1)
                        nc.scalar.activation(sq[:],
                                             x2[:, o * TOK:(o + 1) * TOK],
                                             AF.Square)
                        nc.tensor.matmul(ssq2[:], one_r[:], sq[:],
                                         start=(o == 0), stop=(o == NF - 1))
                    rmsnorm_rs(ssq2)

                    # ================ Phase G/H: norm2 + FFN =============
                    with (
                        tc.tile_pool(name="m4", bufs=1) as m4,
                        tc.tile_pool(name="ps4", bufs=3, space="PSUM") as ps4,
                    ):
                        xn2 = m4.tile([128, NF * TOK], BF16, tag="xn2")
                        for i in range(NF):
                            nc.vector.tensor_mul(
                                xn2[:, i * TOK:(i + 1) * TOK],
                                x2[:, i * TOK:(i + 1) * TOK], rsB[:])
                        # ff1 + silu -> hb (bf16, SBUF resident)
                        hb = m4.tile([128, 64 * TOK], BF16, tag="hb")
                        for o in range(FF // 128):
                            wc = m4.tile([128, D], BF16, tag="wf1c", bufs=2)
                            nc.sync.dma_start(
                                wc[:], wf1p[o * 128:(o + 1) * 128, :])
                            acc = ps4.tile([128, TOK], F32, tag="big")
                            for i in range(NF):
                                nc.tensor.matmul(
                                    acc[:], wc[:, i * 128:(i + 1) * 128],
                                    xn2[:, i * TOK:(i + 1) * TOK],
                                    start=(i == 0), stop=(i == NF - 1))
                            nc.scalar.activation(hb[:, o * TOK:(o + 1) * TOK],
                                                 acc[:], AF.Silu)
                        # ff2 + residual -> outT (weights in 1MB half-chunks)
                        for o in range(NF):
                            acc = ps4.tile([128, TOK], F32, tag="big")
                            for half in range(2):
                                wc2 = m4.tile([128, 32 * 128], BF16,
                                              tag="wf2c", bufs=2)
                                nc.sync.dma_start(
                                    wc2[:],
                                    wf2p[o * 128:(o + 1) * 128,
                                         half * 4096:(half + 1) * 4096])
                                for kk in range(32):
                                    k = half * 32 + kk
                                    nc.tensor.matmul(
                                        acc[:],
                                        wc2[:, kk * 128:(kk + 1) * 128],
                                        hb[:, k * TOK:(k + 1) * TOK],
                                        start=(k == 0), stop=(k == 63))
                            osl = m4.tile([128, TOK], F32, tag="osl", bufs=2)
                            nc.vector.tensor_add(osl[:],
                                                 x2[:, o * TOK:(o + 1) * TOK],
                                                 acc[:])
                            nc.sync.dma_start(outT[o * 128:(o + 1) * 128, :],
                                              osl[:])

    nc.compile()
    return nc


_COMPILED = None


def _own_positions(c):
    c4 = c % 4
    a0, a1 = c4, 7 - c4
    pos = np.r_[256 * a0:256 * a0 + 256, 256 * a1:256 * a1 + 256]
    return pos, a0, a1


def _pack_cols(w, oc):
    """[K, M] -> chunk-major [no*128, nk*oc]: rows o*128+p hold, for output
    column chunk o, the weight rows (i*128+p, o-chunk) laid out (i, m)."""
    k, m = w.shape
    nk, no = k // 128, m // oc
    out = np.empty((no * 128, nk * oc), w.dtype)
    for o in range(no):
        c = w[:, o * oc:(o + 1) * oc].reshape(nk, 128, oc)
        out[o * 128:(o + 1) * 128, :] = (
            c.transpose(1, 0, 2).reshape(128, nk * oc))
    return out


def _prep_inmaps(x, rope_cos, rope_sin, mask, w_norm1, w_norm2, wq, wk, wv,
                 wo, w_ff1, w_ff2):
    x = np.asarray(x, np.float32)
    cos = np.asarray(rope_cos, np.float32)
    sin = np.asarray(rope_sin, np.float32)
    mask = np.asarray(mask)
    wn1 = np.asarray(w_norm1, np.float32)[:, None]
    wn2 = np.asarray(w_norm2, np.float32)[:, None]

    bf = ml_dtypes.bfloat16
    wqp = _pack_cols((wn1 * np.asarray(wq, np.float32)).astype(bf), 128)
    wkp = _pack_cols((wn1 * np.asarray(wk, np.float32)).astype(bf), 128)
    wvp = _pack_cols((wn1 * np.asarray(wv, np.float32)).astype(bf), 256)
    wop = _pack_cols(np.asarray(wo, np.float32).astype(bf), 128)
    wf1p = _pack_cols((wn2 * np.asarray(w_ff1, np.float32)).astype(bf), 128)
    wf2p = _pack_cols(np.asarray(w_ff2, np.float32).astype(bf), 128)

    rr = np.arange(128)[:, None]
    cc = np.arange(256)[None, :]
    diag_lo = (cc >= rr).astype(np.float32)
    diag_hi = (cc >= rr + 128).astype(np.float32)
    zeros = np.zeros((128, 256), np.float32)
    ones = np.ones((128, 256), np.float32)

    in_maps = []
    for c in range(NCORES):
        b = c // 4
        pos, a0, a1 = _own_positions(c)
        s = sin[pos].T.copy()
        s2 = np.concatenate([-s[:64], s[64:]], axis=0)

        # multiplicative mask tiles: t<8 -> slot 0 (q-block a0), else slot 1
        tiles = []
        for t in range(16):
            a = a0 if t < 8 else a1
            ext = 2 * a + 2
            if t == ext - 2:
                tiles.append(diag_lo)
            elif t == ext - 1:
                tiles.append(diag_hi)
            elif t >= ext:
                tiles.append(zeros)
            else:
                tiles.append(ones)
        maskM = np.concatenate(tiles, axis=1).astype(bf)

        # key-padding bias per kv block: col j <- kv pos 128*j + r
        mb = np.where(mask[b] != 0, 0.0, NEG).astype(np.float32)  # [L]
        mbias = np.ascontiguousarray(mb.reshape(16, 128).T)

        in_maps.append({
            "xT": np.ascontiguousarray(x[b, pos].T),
            "wqp": wqp, "wkp": wkp, "wvp": wvp, "wop": wop,
            "wf1p": wf1p, "wf2p": wf2p,
            "ropeC": np.ascontiguousarray(cos[pos].T),
            "ropeS2": np.ascontiguousarray(s2),
            "maskM": np.ascontiguousarray(maskM),
            "mbias": mbias,
            "onesf": np.ones((128, 1), np.float32),
            "onesb": np.ones((128, 1), ml_dtypes.bfloat16),
        })
    return in_maps


def _assemble(res):
    out = np.empty((B, L, D), np.float32)
    for c in range(NCORES):
        b = c // 4
        pos, _, _ = _own_positions(c)
        out[b, pos, :] = res.results[c]["outT"].T
    return out


def kernel(**inputs):
    global _COMPILED
    if _COMPILED is None:
        _COMPILED = _build()
    in_maps = _prep_inmaps(**inputs)
    res = run_bass_kernel_spmd(_COMPILED, in_maps, list(range(NCORES)))
    return _assemble(res)


def timed_run(**inputs):
    """Run with NTFF profiling; returns (exec_time_ns, BassKernelResults)."""
    global _COMPILED
    if _COMPILED is None:
        _COMPILED = _build()
    in_maps = _prep_inmaps(**inputs)
    res = run_bass_kernel_spmd(_COMPILED, in_maps, list(range(NCORES)),
                               trace=True)
    return res.exec_time_ns, res
